# revision 52
# baseline (speedup 1.0000x reference)
"""Trainium2 Bass kernel for nn_MultiHeadAttention_4810363372776 (linear attention).

Sharding: data-parallel over batch (4) x tensor-parallel over head groups (2).
Core i handles batch i//2, heads [8*(i%2), 8*(i%2)+8). Each core computes its
partial output projection; the host sums the two head-group partials per batch
and adds the output bias.

q/k/v are transposed to [d, s] and packed on the host (removes all on-device
PE transposes of x; every DMA is a contiguous 1-4KB-per-partition block). The
exp-damped q/k path runs fp8 DoubleRow (xq, xk, Wq, Wk — quantization errors
enter the exponent scaled by 1/8 and the softmax normalizer cancels); the
linear v path (xv, Wv, ctx, Wo) stays bf16 since fp8 there costs ~4% output
error each. Output is written bf16 and upcast on the host, which also adds bo
during the head-group pair-sum. Weight/x DMAs are interleaved across the
sync/scalar/gpsimd queues at startup; phase 2 is software-pipelined
(num -> ctx -> ctxT -> out-proj with 2/4-tile lags).
"""

import functools
import numpy as np

B, S, D, H = 4, 4096, 1024, 16
DK = D // H          # 64
OG = D // 2          # 512 per-core head-group width (8 heads)
NCORES = 8
SCALE = 1.0 / 8.0    # 1/sqrt(DK)
NT = S // 128        # 32 s-tiles
SM = 512             # q-proj macro (4 s-tiles)
NMAC = S // SM       # 8 macros


@functools.lru_cache(maxsize=2)
def _build(kv_bias=False):
    import concourse.bass as bass  # noqa: F401
    from concourse import bacc
    import concourse.mybir as mybir
    import concourse.tile as tile
    from concourse.masks import make_identity
    from contextlib import ExitStack

    f32 = mybir.dt.float32
    bf16 = mybir.dt.bfloat16
    fp8 = mybir.dt.float8e4
    DR = mybir.MatmulPerfMode.DoubleRow
    EXP = mybir.ActivationFunctionType.Exp
    COPY = mybir.ActivationFunctionType.Copy
    AXX = mybir.AxisListType.X
    ADD = mybir.AluOpType.add

    nc = bacc.Bacc()

    # x pre-transposed+packed on host: row st*128+p holds [t*128+s_local] with
    # d = t*128 + p.
    xkp = nc.declare_dram_parameter("xkp", [NT * 128, D], fp8, isOutput=False)
    xvp = nc.declare_dram_parameter("xvp", [NT * 128, D], bf16, isOutput=False)
    # q packed per macro: row a*128+p holds [t*512+s_local]
    xqp = nc.declare_dram_parameter("xqp", [NMAC * 128, 8 * SM], fp8, isOutput=False)
    wqt = nc.declare_dram_parameter("wqt", [D, OG], fp8, isOutput=False)
    wkt = nc.declare_dram_parameter("wkt", [D, OG], fp8, isOutput=False)
    wvt = nc.declare_dram_parameter("wvt", [D, OG], bf16, isOutput=False)
    wot = nc.declare_dram_parameter("wot", [OG, D], bf16, isOutput=False)
    bqsp = nc.declare_dram_parameter("bqs", [128, 4], f32, isOutput=False)
    bkp = nc.declare_dram_parameter("bk", [1, OG], f32, isOutput=False)
    bvp = nc.declare_dram_parameter("bv", [1, OG], f32, isOutput=False)
    maskp = nc.declare_dram_parameter("maskf", [128, NT], f32, isOutput=False)
    out = nc.declare_dram_parameter("out", [NT * 128, D], bf16, isOutput=True)

    with tile.TileContext(nc) as tc:
        with ExitStack() as ctx:
            singles = ctx.enter_context(tc.tile_pool(name="singles", bufs=1))

            ident = singles.tile([128, 128], bf16)
            make_identity(nc, ident)
            # block-diag [kv | ksum] per head pair; zeroed here so the
            # phase-1 -> phase-2 boundary only does the small copies
            kvbd = [singles.tile([128, 130], bf16, tag=f"kvbd{p}", name=f"kvbd{p}") for p in range(4)]
            for p in range(4):
                nc.vector.memset(kvbd[p], 0.0)

            # weights needed first (k proj of s-tile 0): wk split in 4 tiles so
            # the first matmul can start after 128KB; spread the startup DMA
            # burst across sync/scalar/vector/gpsimd queues.
            wk_sb = [singles.tile([128, 2, OG], fp8, tag=f"wk{t2}", name=f"wk{t2}") for t2 in range(4)]
            mask_sb = singles.tile([128, NT], f32, tag="mask")
            wq_sb = singles.tile([128, 8, OG], fp8, tag="wq")
            bqs_sb = singles.tile([128, 4], f32, tag="bqs")
            wv_sb = [singles.tile([128, 1, OG], bf16, tag=f"wv{t}", name=f"wv{t}") for t in range(8)]
            wo_sb = singles.tile([128, 4, D], bf16, tag="wo")
            if kv_bias:
                bk_bc = singles.tile([128, OG], f32, tag="bk_bc")
                nc.gpsimd.dma_start(out=bk_bc, in_=bkp[:, :].partition_broadcast(128))
                bv_bc = singles.tile([128, OG], f32, tag="bv_bc")
                nc.gpsimd.dma_start(out=bv_bc, in_=bvp[:, :].partition_broadcast(128))

            # exp(q_hat * scale), stored [o (4 blocks of 128 = head pairs), s]
            ET = singles.tile([128, 4, S], bf16, tag="ET")

            # phase-2 numerator PSUM lives in the 2 banks phase 1 leaves free,
            # so the first num matmuls don't wait on phase-1 pool teardown
            pnum_pool = ctx.enter_context(tc.tile_pool(name="pnum", bufs=1, space="PSUM"))

            # ---------------- phase 1 ----------------
            with ExitStack() as p1:
                pacc_pool = p1.enter_context(tc.tile_pool(name="pacc", bufs=1, space="PSUM"))
                # two chains per bank; bank-wide has_written clear happens once (st==0, even pair)
                kvps = [pacc_pool.tile([128, 2, 129], f32, tag=f"kvacc{i}", name=f"kvacc{i}") for i in range(2)]
                xk_pool = p1.enter_context(tc.tile_pool(name="xk", bufs=6))
                xv_pool = p1.enter_context(tc.tile_pool(name="xv", bufs=6))
                xq_pool = p1.enter_context(tc.tile_pool(name="xq", bufs=2))
                ek_pool = p1.enter_context(tc.tile_pool(name="ek", bufs=3))
                kf_pool = p1.enter_context(tc.tile_pool(name="kf", bufs=7))
                vf_pool = p1.enter_context(tc.tile_pool(name="vf", bufs=3))
                pkv_pool = p1.enter_context(tc.tile_pool(name="pkv", bufs=4, space="PSUM"))


                def flush_kv(pending):
                    kf, vf, pst = pending
                    for p in range(4):
                        nc.tensor.matmul(
                            kvps[p // 2][:, p % 2, 0:129],
                            kf[:, 2 * p:2 * p + 2, :],
                            vf[:, p, 0:129],
                            start=(pst == 0 and p % 2 == 0),
                            stop=(pst == NT - 1),
                            skip_group_check=True,
                        )

                xk_tiles = {}
                xv_tiles = {}
                xq_tiles = {}

                def fetch_k(st, q=None):
                    t = xk_pool.tile([128, 8, 128], fp8, tag="xk")
                    (q or nc.gpsimd).dma_start(out=t, in_=xkp[st * 128:(st + 1) * 128, :].rearrange("p (t s) -> p t s", s=128))
                    xk_tiles[st] = t

                def fetch_v(st, q=None):
                    t = xv_pool.tile([128, 8, 128], bf16, tag="xv")
                    (q or nc.sync).dma_start(out=t, in_=xvp[st * 128:(st + 1) * 128, :].rearrange("p (t s) -> p t s", s=128))
                    xv_tiles[st] = t

                def fetch(st):
                    fetch_k(st)
                    fetch_v(st)

                def fetch_q(a):
                    t = xq_pool.tile([128, 8, SM], fp8, tag="xq")
                    nc.gpsimd.dma_start(out=t, in_=xqp[a * 128:(a + 1) * 128, :].rearrange("p (t s) -> p t s", s=SM))
                    xq_tiles[a] = t

                PF = 5  # s-tiles of prefetch depth
                pend = [None]
                # startup: interleave weight chunks with the first x tiles on
                # each queue so PE can start after ~256KB, not ~2.5MB.
                nc.scalar.dma_start(out=mask_sb, in_=maskp[:, :])
                fetch_k(0, q=nc.gpsimd)
                nc.sync.dma_start(out=wk_sb[0], in_=wkt[0:256, :].rearrange("(t p) o -> p t o", p=128))
                fetch_v(0, q=nc.scalar)
                fetch_k(1, q=nc.gpsimd)
                nc.sync.dma_start(out=wk_sb[1], in_=wkt[256:512, :].rearrange("(t p) o -> p t o", p=128))
                for t in range(4):
                    nc.scalar.dma_start(out=wv_sb[t], in_=wvt[128 * t:128 * (t + 1), :].rearrange("(t p) o -> p t o", p=128))
                nc.sync.dma_start(out=wk_sb[2], in_=wkt[512:768, :].rearrange("(t p) o -> p t o", p=128))
                fetch_k(2, q=nc.gpsimd)
                nc.sync.dma_start(out=wk_sb[3], in_=wkt[768:1024, :].rearrange("(t p) o -> p t o", p=128))
                for t in range(4, 8):
                    nc.scalar.dma_start(out=wv_sb[t], in_=wvt[128 * t:128 * (t + 1), :].rearrange("(t p) o -> p t o", p=128))
                fetch_v(1, q=nc.sync)
                fetch_q(0)
                fetch_v(2, q=nc.sync)
                nc.scalar.dma_start(out=bqs_sb, in_=bqsp[:, :])
                # q-path weights are not needed until the end of macro 0
                nc.gpsimd.dma_start(out=wq_sb, in_=wqt[:, :].rearrange("(t p) o -> p t o", p=128))
                for st in range(3, PF):
                    fetch(st)

                for a in range(NMAC):
                    if a + 1 < NMAC:
                        fetch_q(a + 1)
                    if a == 1:
                        # phase-2 weights: load while phase 1 runs
                        nc.sync.dma_start(out=wo_sb, in_=wot[:, :].rearrange("(t p) o -> p t o", p=128))
                    for u in range(SM // 128):
                        st = a * (SM // 128) + u
                        if st + PF < NT:
                            fetch(st + PF)
                        xkt = xk_tiles.pop(st)
                        xvt = xv_tiles.pop(st)

                        # k projection (fp8 DoubleRow)
                        pk = pkv_pool.tile([128, OG], f32, tag="pkv")
                        for t2 in range(4):
                            nc.tensor.matmul(pk, xkt[:, 2 * t2:2 * t2 + 2, :],
                                             wk_sb[t2][:, :, :],
                                             start=(t2 == 0), stop=(t2 == 3), perf_mode=DR)
                        if kv_bias:
                            nc.vector.tensor_add(pk, pk, bk_bc)
                        ek = ek_pool.tile([128, OG], bf16, tag="ek")
                        nc.scalar.activation(ek, pk, EXP, scale=SCALE)
                        rows = ek_pool.tile([128, 8], f32, tag="rows")
                        nc.vector.tensor_reduce(rows, ek.rearrange("p (h e) -> p h e", h=8), axis=AXX, op=ADD)
                        nc.vector.reciprocal(rows, rows)
                        nc.vector.tensor_scalar_mul(rows, rows, mask_sb[:, st:st + 1])
                        kf = kf_pool.tile([128, 8, DK], bf16, tag="kf")
                        nc.vector.tensor_mul(
                            kf,
                            ek.rearrange("p (h e) -> p h e", h=8),
                            rows[:, :, None].to_broadcast([128, 8, DK]),
                        )

                        # v projection (bf16)
                        pv = pkv_pool.tile([128, OG], f32, tag="pkv")
                        for t in range(8):
                            nc.tensor.matmul(pv, xvt[:, t, :], wv_sb[t][:, 0, :], start=(t == 0), stop=(t == 7))
                        if kv_bias:
                            nc.vector.tensor_add(pv, pv, bv_bc)
                        vf = vf_pool.tile([128, 4, 130], bf16, tag="vf")
                        nc.scalar.activation(vf[:, :, 0:128], pv.rearrange("p (j s) -> p j s", j=4), COPY, scale=mask_sb[:, st:st + 1])
                        nc.vector.memset(vf[:, :, 128:129], 1.0)

                        # deferred kv accumulation for the previous s-tile
                        if pend[0] is not None:
                            flush_kv(pend[0])
                        pend[0] = (kf, vf, st)
                        if st == NT - 1:
                            # flush before the last q-proj so kvbd building
                            # overlaps it
                            flush_kv(pend[0])
                            pend[0] = None

                    # q projection for the macro, output transposed [o, s]
                    xq_sb = xq_tiles.pop(a)
                    for ob in range(4):
                        pq = pkv_pool.tile([128, SM], f32, tag="pkv")
                        for t2 in range(4):
                            nc.tensor.matmul(pq, wq_sb[:, 2 * t2:2 * t2 + 2, ob * 128:(ob + 1) * 128],
                                             xq_sb[:, 2 * t2:2 * t2 + 2, :],
                                             start=(t2 == 0), stop=(t2 == 3), perf_mode=DR)
                        nc.scalar.activation(ET[:, ob, a * SM:(a + 1) * SM], pq, EXP, bias=bqs_sb[:, ob:ob + 1], scale=SCALE)

                if pend[0] is not None:
                    flush_kv(pend[0])

                # build block-diag [kv | ksum] tiles (bf16); memset already done.
                # Split the copies across DVE and ACT so the chain is shorter.
                for p in range(4):
                    ps = kvps[p // 2][:, p % 2]
                    eng = nc.vector if p % 2 == 0 else None
                    if eng is not None:
                        eng.tensor_copy(kvbd[p][0:64, 0:64], ps[0:64, 0:64])
                        eng.tensor_copy(kvbd[p][0:64, 64:65], ps[0:64, 128:129])
                        eng.tensor_copy(kvbd[p][64:128, 65:129], ps[64:128, 64:128])
                        eng.tensor_copy(kvbd[p][64:128, 129:130], ps[64:128, 128:129])
                    else:
                        nc.scalar.copy(out=kvbd[p][0:64, 0:64], in_=ps[0:64, 0:64])
                        nc.scalar.copy(out=kvbd[p][0:64, 64:65], in_=ps[0:64, 128:129])
                        nc.scalar.copy(out=kvbd[p][64:128, 65:129], in_=ps[64:128, 64:128])
                        nc.scalar.copy(out=kvbd[p][64:128, 129:130], in_=ps[64:128, 128:129])

            # ---------------- phase 2 ----------------
            # stages per s-tile: num -> (DVE) ctx -> (PE) ctxT -> (ACT) evac -> (PE) out-proj
            # software-pipelined: ctxT lags one tile, out-proj lags two.
            with ExitStack() as p2s:
                p2 = p2s.enter_context(tc.tile_pool(name="p2", bufs=4))
                pct_pool = p2s.enter_context(tc.tile_pool(name="pct", bufs=2, space="PSUM"))
                po_pool = p2s.enter_context(tc.tile_pool(name="po", bufs=3, space="PSUM"))

                ctx_q = {}   # st -> ctx tile
                ctxT_q = {}  # st -> ctxT tile

                def stage_num(st):
                    s0 = st * 128
                    pnums = [pnum_pool.tile([128, 2, 130], f32, tag=f"pnum{i}", name=f"pnum{i}") for i in range(2)]
                    for p in range(4):
                        nc.tensor.matmul(pnums[p // 2][:, p % 2, :], ET[:, p, s0:s0 + 128], kvbd[p], start=True, stop=True)
                    ctxs = p2.tile([128, OG], bf16, tag="ctx", name="ctxs")
                    for i in range(2):
                        pn4 = pnums[i].rearrange("p j (two c) -> p (j two) c", two=2)  # [128, 4, 65]
                        r4 = p2.tile([128, 4, 1], f32, tag="r", name="r4")
                        nc.vector.reciprocal(r4, pn4[:, :, 64:65])
                        ctx4 = ctxs[:, i * 256:(i + 1) * 256].rearrange("p (j c) -> p j c", c=64)
                        nc.vector.tensor_mul(ctx4, pn4[:, :, 0:64], r4.to_broadcast([128, 4, 64]))
                    ctx_q[st] = ctxs

                def stage_ctxT(st):
                    ctxs = ctx_q.pop(st)
                    pct = pct_pool.tile([128, 512], bf16, tag="pct", name="pct")
                    for eb in range(4):
                        nc.tensor.transpose(pct[:, eb * 128:(eb + 1) * 128], ctxs[:, eb * 128:(eb + 1) * 128], ident)
                    ctxT = p2.tile([128, 4, 128], bf16, tag="ctxT", name="ctxT")
                    nc.scalar.copy(out=ctxT, in_=pct.rearrange("p (j s) -> p j s", j=4))
                    ctxT_q[st] = ctxT

                def stage_oproj(st):
                    ctxT = ctxT_q.pop(st)
                    for half in range(2):
                        po = po_pool.tile([128, 512], f32, tag="po", name="po")
                        for eb in range(4):
                            nc.tensor.matmul(po, ctxT[:, eb, :],
                                             wo_sb[:, eb, half * 512:(half + 1) * 512],
                                             start=(eb == 0), stop=(eb == 3))
                        outsb = p2.tile([128, 512], bf16, tag=f"outsb{half}", name="outsb")
                        if half == 0:
                            nc.scalar.copy(out=outsb, in_=po)
                            dq = nc.sync
                        else:
                            nc.vector.tensor_copy(outsb, po)
                            dq = nc.gpsimd  # gpsimd queue is idle in phase 2
                        dq.dma_start(out=out[st * 128:(st + 1) * 128, half * 512:(half + 1) * 512], in_=outsb)

                for st in range(NT):
                    stage_num(st)
                    if st >= 2:
                        stage_ctxT(st - 2)
                    if st >= 4:
                        stage_oproj(st - 4)
                for st in range(NT - 2, NT):
                    stage_ctxT(st)
                for st in range(NT - 4, NT):
                    stage_oproj(st)

    nc.compile()
    return nc


_LAST_RESULT = None


def _pack_st(x, dt_):
    # [S, D] f32 -> [NT*128, 1024] dt, row st*128+p col t*128+s_local = x[st*128+s, t*128+p]
    xr = x.reshape(NT, 128, 8, 128).transpose(0, 3, 2, 1)  # [st, p, t, s]
    return np.ascontiguousarray(xr.reshape(NT * 128, D)).astype(dt_)


def _pack_q(q, f8):
    # [S, D] f32 -> [NMAC*128, 4096] fp8, row a*128+p col t*512+s_local
    qr = q.reshape(NMAC, SM, 8, 128).transpose(0, 3, 2, 1)  # [a, p, t, s]
    return np.ascontiguousarray(qr.reshape(NMAC * 128, 8 * SM)).astype(f8)


def kernel(q, k, v, mask, Wq, bq, Wk, bk, Wv, bv, Wo, bo):
    global _LAST_RESULT
    import ml_dtypes
    from concourse.bass_utils import run_bass_kernel_spmd

    q = np.asarray(q, np.float32)
    k = np.asarray(k, np.float32)
    v = np.asarray(v, np.float32)
    mask = np.asarray(mask)
    Wq = np.asarray(Wq, np.float32)
    Wk = np.asarray(Wk, np.float32)
    Wv = np.asarray(Wv, np.float32)
    Wo = np.asarray(Wo, np.float32)
    bq = np.asarray(bq, np.float32)
    bk = np.asarray(bk, np.float32)
    bv = np.asarray(bv, np.float32)
    bo = np.asarray(bo, np.float32)

    nc = _build(bool(np.any(bk) or np.any(bv)))

    f8 = ml_dtypes.float8_e4m3
    bf = ml_dtypes.bfloat16
    xk_b = [_pack_st(k[b], f8) for b in range(B)]
    xv_b = [_pack_st(v[b], bf) for b in range(B)]
    xq_b = [_pack_q(q[b], f8) for b in range(B)]

    in_maps = []
    for core in range(NCORES):
        b, g = core // 2, core % 2
        sl = slice(g * OG, (g + 1) * OG)
        maskf = mask[b, 0, 0, :].astype(np.float32).reshape(NT, 128).T.copy()
        in_maps.append({
            "xkp": xk_b[b],
            "xvp": xv_b[b],
            "xqp": xq_b[b],
            "wqt": np.ascontiguousarray(Wq[sl, :].T).astype(f8),
            "wkt": np.ascontiguousarray(Wk[sl, :].T).astype(f8),
            "wvt": np.ascontiguousarray(Wv[sl, :].T).astype(bf),
            "wot": np.ascontiguousarray(Wo[:, sl].T).astype(bf),
            "bqs": np.ascontiguousarray((bq[sl] * SCALE).reshape(4, 128).T),
            "bk": bk[sl].reshape(1, OG).copy(),
            "bv": bv[sl].reshape(1, OG).copy(),
            "maskf": maskf,
        })

    res = run_bass_kernel_spmd(nc, in_maps, list(range(NCORES)))
    _LAST_RESULT = res

    outp = np.empty((B, S, D), np.float32)
    for b in range(B):
        o0 = res.results[2 * b]["out"].astype(np.float32).reshape(S, D)
        o1 = res.results[2 * b + 1]["out"].astype(np.float32).reshape(S, D)
        outp[b] = o0 + o1 + bo[None, :]
    return outp


# revision 55
# speedup vs baseline: 1.0371x; 1.0371x over previous
"""Trainium2 Bass kernel for nn_MultiHeadAttention_4810363372776 (linear attention).

Sharding: data-parallel over batch (4) x tensor-parallel over head groups (2).
Core i handles batch i//2, heads [8*(i%2), 8*(i%2)+8). Each core computes its
partial output projection; the host sums the two head-group partials per batch
and adds the output bias.

q/k/v are transposed to [d, s] and packed on the host (removes all on-device
PE transposes of x; every DMA is a contiguous 1-4KB-per-partition block). The
exp-damped q/k path runs fp8 DoubleRow (xq, xk, Wq, Wk — quantization errors
enter the exponent scaled by 1/8 and the softmax normalizer cancels); the
linear v path (xv, Wv, ctx, Wo) stays bf16 since fp8 there costs ~4% output
error each. Output is written bf16 and upcast on the host, which also adds bo
during the head-group pair-sum. Weight/x DMAs are interleaved across the
sync/scalar/gpsimd queues at startup; phase 2 is software-pipelined
(num -> ctx -> ctxT -> out-proj with 2/4-tile lags).
"""

import functools
import numpy as np

B, S, D, H = 4, 4096, 1024, 16
DK = D // H          # 64
OG = D // 2          # 512 per-core head-group width (8 heads)
NCORES = 8
SCALE = 1.0 / 8.0    # 1/sqrt(DK)
NT = S // 128        # 32 s-tiles
SM = 512             # q-proj macro (4 s-tiles)
NMAC = S // SM       # 8 macros


@functools.lru_cache(maxsize=2)
def _build(kv_bias=False):
    import concourse.bass as bass  # noqa: F401
    from concourse import bacc
    import concourse.mybir as mybir
    import concourse.tile as tile
    from concourse.masks import make_identity
    from contextlib import ExitStack

    f32 = mybir.dt.float32
    bf16 = mybir.dt.bfloat16
    fp8 = mybir.dt.float8e4
    DR = mybir.MatmulPerfMode.DoubleRow
    EXP = mybir.ActivationFunctionType.Exp
    COPY = mybir.ActivationFunctionType.Copy
    AXX = mybir.AxisListType.X
    ADD = mybir.AluOpType.add

    nc = bacc.Bacc()

    # x pre-transposed+packed on host: row st*128+p holds [t*128+s_local] with
    # d = t*128 + p.
    xkp = nc.declare_dram_parameter("xkp", [NT * 128, D], fp8, isOutput=False)
    xvp = nc.declare_dram_parameter("xvp", [NT * 128, D], bf16, isOutput=False)
    # q packed per macro: row a*128+p holds [t*512+s_local]
    xqp = nc.declare_dram_parameter("xqp", [NMAC * 128, 8 * SM], fp8, isOutput=False)
    wqt = nc.declare_dram_parameter("wqt", [D, OG], fp8, isOutput=False)
    wkt = nc.declare_dram_parameter("wkt", [D, OG], fp8, isOutput=False)
    wvt = nc.declare_dram_parameter("wvt", [D, OG], bf16, isOutput=False)
    wot = nc.declare_dram_parameter("wot", [OG, D], bf16, isOutput=False)
    bqsp = nc.declare_dram_parameter("bqs", [128, 4], f32, isOutput=False)
    bkp = nc.declare_dram_parameter("bk", [1, OG], f32, isOutput=False)
    bvp = nc.declare_dram_parameter("bv", [1, OG], f32, isOutput=False)
    maskp = nc.declare_dram_parameter("maskf", [128, NT], f32, isOutput=False)
    out = nc.declare_dram_parameter("out", [NT * 128, D], bf16, isOutput=True)

    with tile.TileContext(nc) as tc:
        with ExitStack() as ctx:
            singles = ctx.enter_context(tc.tile_pool(name="singles", bufs=1))

            ident = singles.tile([128, 128], bf16)
            make_identity(nc, ident)
            # block-diag [kv | ksum] per head pair; zeroed here so the
            # phase-1 -> phase-2 boundary only does the small copies
            kvbd = [singles.tile([128, 130], bf16, tag=f"kvbd{p}", name=f"kvbd{p}") for p in range(4)]
            for p in range(4):
                nc.vector.memset(kvbd[p], 0.0)

            # weights needed first (k proj of s-tile 0): wk split in 4 tiles so
            # the first matmul can start after 128KB; spread the startup DMA
            # burst across sync/scalar/vector/gpsimd queues.
            wk_sb = [singles.tile([128, 2, OG], fp8, tag=f"wk{t2}", name=f"wk{t2}") for t2 in range(4)]
            mask_sb = singles.tile([128, NT], f32, tag="mask")
            wq_sb = singles.tile([128, 8, OG], fp8, tag="wq")
            bqs_sb = singles.tile([128, 4], f32, tag="bqs")
            wv_sb = [singles.tile([128, 1, OG], bf16, tag=f"wv{t}", name=f"wv{t}") for t in range(8)]
            wo_sb = singles.tile([128, 4, D], bf16, tag="wo")
            if kv_bias:
                bk_bc = singles.tile([128, OG], f32, tag="bk_bc")
                nc.gpsimd.dma_start(out=bk_bc, in_=bkp[:, :].partition_broadcast(128))
                bv_bc = singles.tile([128, OG], f32, tag="bv_bc")
                nc.gpsimd.dma_start(out=bv_bc, in_=bvp[:, :].partition_broadcast(128))

            # exp(q_hat * scale), stored [o (4 blocks of 128 = head pairs), s]
            ET = singles.tile([128, 4, S], bf16, tag="ET")



            # ---------------- phase 1 ----------------
            with ExitStack() as p1:
                pacc_pool = p1.enter_context(tc.tile_pool(name="pacc", bufs=1, space="PSUM"))
                # two chains per bank; bank-wide has_written clear happens once (st==0, even pair)
                kvps = [pacc_pool.tile([128, 2, 129], f32, tag=f"kvacc{i}", name=f"kvacc{i}") for i in range(2)]
                xk_pool = p1.enter_context(tc.tile_pool(name="xk", bufs=6))
                xv_pool = p1.enter_context(tc.tile_pool(name="xv", bufs=6))
                xq_pool = p1.enter_context(tc.tile_pool(name="xq", bufs=2))
                ek_pool = p1.enter_context(tc.tile_pool(name="ek", bufs=3))
                kf_pool = p1.enter_context(tc.tile_pool(name="kf", bufs=7))
                vf_pool = p1.enter_context(tc.tile_pool(name="vf", bufs=3))
                pkv_pool = p1.enter_context(tc.tile_pool(name="pkv", bufs=4, space="PSUM"))


                def flush_kv(pending):
                    kf, vf, pst = pending
                    for p in range(4):
                        nc.tensor.matmul(
                            kvps[p // 2][:, p % 2, 0:129],
                            kf[:, 2 * p:2 * p + 2, :],
                            vf[:, p, 0:129],
                            start=(pst == 0 and p % 2 == 0),
                            stop=(pst == NT - 1),
                            skip_group_check=True,
                        )

                xk_tiles = {}
                xv_tiles = {}
                xq_tiles = {}

                def fetch_k(st, q=None):
                    t = xk_pool.tile([128, 8, 128], fp8, tag="xk")
                    (q or nc.gpsimd).dma_start(out=t, in_=xkp[st * 128:(st + 1) * 128, :].rearrange("p (t s) -> p t s", s=128))
                    xk_tiles[st] = t

                def fetch_v(st, q=None):
                    t = xv_pool.tile([128, 8, 128], bf16, tag="xv")
                    (q or nc.sync).dma_start(out=t, in_=xvp[st * 128:(st + 1) * 128, :].rearrange("p (t s) -> p t s", s=128))
                    xv_tiles[st] = t

                def fetch(st):
                    fetch_k(st)
                    fetch_v(st)

                def fetch_q(a):
                    t = xq_pool.tile([128, 8, SM], fp8, tag="xq")
                    nc.gpsimd.dma_start(out=t, in_=xqp[a * 128:(a + 1) * 128, :].rearrange("p (t s) -> p t s", s=SM))
                    xq_tiles[a] = t

                PF = 5  # s-tiles of prefetch depth
                pend = [None]
                # startup: interleave weight chunks with the first x tiles on
                # each queue so PE can start after ~256KB, not ~2.5MB.
                nc.scalar.dma_start(out=mask_sb, in_=maskp[:, :])
                fetch_k(0, q=nc.gpsimd)
                nc.sync.dma_start(out=wk_sb[0], in_=wkt[0:256, :].rearrange("(t p) o -> p t o", p=128))
                fetch_v(0, q=nc.scalar)
                fetch_k(1, q=nc.gpsimd)
                nc.sync.dma_start(out=wk_sb[1], in_=wkt[256:512, :].rearrange("(t p) o -> p t o", p=128))
                for t in range(4):
                    nc.scalar.dma_start(out=wv_sb[t], in_=wvt[128 * t:128 * (t + 1), :].rearrange("(t p) o -> p t o", p=128))
                nc.sync.dma_start(out=wk_sb[2], in_=wkt[512:768, :].rearrange("(t p) o -> p t o", p=128))
                fetch_k(2, q=nc.gpsimd)
                nc.sync.dma_start(out=wk_sb[3], in_=wkt[768:1024, :].rearrange("(t p) o -> p t o", p=128))
                for t in range(4, 8):
                    nc.scalar.dma_start(out=wv_sb[t], in_=wvt[128 * t:128 * (t + 1), :].rearrange("(t p) o -> p t o", p=128))
                fetch_v(1, q=nc.sync)
                fetch_q(0)
                fetch_v(2, q=nc.sync)
                nc.scalar.dma_start(out=bqs_sb, in_=bqsp[:, :])
                # q-path weights are not needed until the end of macro 0
                nc.gpsimd.dma_start(out=wq_sb, in_=wqt[:, :].rearrange("(t p) o -> p t o", p=128))
                for st in range(3, PF):
                    fetch(st)

                for a in range(NMAC):
                    if a + 1 < NMAC:
                        fetch_q(a + 1)
                    if a == 1:
                        # phase-2 weights: load while phase 1 runs
                        nc.sync.dma_start(out=wo_sb, in_=wot[:, :].rearrange("(t p) o -> p t o", p=128))
                    for u in range(SM // 128):
                        st = a * (SM // 128) + u
                        if st + PF < NT:
                            fetch(st + PF)
                        xkt = xk_tiles.pop(st)
                        xvt = xv_tiles.pop(st)

                        # k projection (fp8 DoubleRow)
                        pk = pkv_pool.tile([128, OG], f32, tag="pkv")
                        for t2 in range(4):
                            nc.tensor.matmul(pk, xkt[:, 2 * t2:2 * t2 + 2, :],
                                             wk_sb[t2][:, :, :],
                                             start=(t2 == 0), stop=(t2 == 3), perf_mode=DR)
                        if kv_bias:
                            nc.vector.tensor_add(pk, pk, bk_bc)
                        ek = ek_pool.tile([128, OG], bf16, tag="ek")
                        nc.scalar.activation(ek, pk, EXP, scale=SCALE)
                        rows = ek_pool.tile([128, 8], f32, tag="rows")
                        nc.vector.tensor_reduce(rows, ek.rearrange("p (h e) -> p h e", h=8), axis=AXX, op=ADD)
                        nc.vector.reciprocal(rows, rows)
                        nc.vector.tensor_scalar_mul(rows, rows, mask_sb[:, st:st + 1])
                        kf = kf_pool.tile([128, 8, DK], bf16, tag="kf")
                        nc.vector.tensor_mul(
                            kf,
                            ek.rearrange("p (h e) -> p h e", h=8),
                            rows[:, :, None].to_broadcast([128, 8, DK]),
                        )

                        # v projection (bf16)
                        pv = pkv_pool.tile([128, OG], f32, tag="pkv")
                        for t in range(8):
                            nc.tensor.matmul(pv, xvt[:, t, :], wv_sb[t][:, 0, :], start=(t == 0), stop=(t == 7))
                        if kv_bias:
                            nc.vector.tensor_add(pv, pv, bv_bc)
                        vf = vf_pool.tile([128, 4, 130], bf16, tag="vf")
                        nc.scalar.activation(vf[:, :, 0:128], pv.rearrange("p (j s) -> p j s", j=4), COPY, scale=mask_sb[:, st:st + 1])
                        nc.vector.memset(vf[:, :, 128:129], 1.0)

                        # deferred kv accumulation for the previous s-tile
                        if pend[0] is not None:
                            flush_kv(pend[0])
                        pend[0] = (kf, vf, st)
                        if st == NT - 1:
                            # flush before the last q-proj so kvbd building
                            # overlaps it
                            flush_kv(pend[0])
                            pend[0] = None

                    # q projection for the macro, output transposed [o, s]
                    xq_sb = xq_tiles.pop(a)
                    for ob in range(4):
                        pq = pkv_pool.tile([128, SM], f32, tag="pkv")
                        for t2 in range(4):
                            nc.tensor.matmul(pq, wq_sb[:, 2 * t2:2 * t2 + 2, ob * 128:(ob + 1) * 128],
                                             xq_sb[:, 2 * t2:2 * t2 + 2, :],
                                             start=(t2 == 0), stop=(t2 == 3), perf_mode=DR)
                        nc.scalar.activation(ET[:, ob, a * SM:(a + 1) * SM], pq, EXP, bias=bqs_sb[:, ob:ob + 1], scale=SCALE)

                if pend[0] is not None:
                    flush_kv(pend[0])

                # build block-diag [kv | ksum] tiles (bf16); memset already done.
                # Split the copies across DVE and ACT so the chain is shorter.
                for p in range(4):
                    ps = kvps[p // 2][:, p % 2]
                    eng = nc.vector if p % 2 == 0 else None
                    if eng is not None:
                        eng.tensor_copy(kvbd[p][0:64, 0:64], ps[0:64, 0:64])
                        eng.tensor_copy(kvbd[p][0:64, 64:65], ps[0:64, 128:129])
                        eng.tensor_copy(kvbd[p][64:128, 65:129], ps[64:128, 64:128])
                        eng.tensor_copy(kvbd[p][64:128, 129:130], ps[64:128, 128:129])
                    else:
                        nc.scalar.copy(out=kvbd[p][0:64, 0:64], in_=ps[0:64, 0:64])
                        nc.scalar.copy(out=kvbd[p][0:64, 64:65], in_=ps[0:64, 128:129])
                        nc.scalar.copy(out=kvbd[p][64:128, 65:129], in_=ps[64:128, 64:128])
                        nc.scalar.copy(out=kvbd[p][64:128, 129:130], in_=ps[64:128, 128:129])

            # ---------------- phase 2 ----------------
            # stages per s-tile: num -> (DVE) ctx -> (PE) ctxT -> (ACT) evac -> (PE) out-proj
            # software-pipelined: ctxT lags one tile, out-proj lags two.
            with ExitStack() as p2s:
                p2 = p2s.enter_context(tc.tile_pool(name="p2", bufs=4))
                pnum_pool = p2s.enter_context(tc.tile_pool(name="pnum", bufs=2, space="PSUM"))
                pct_pool = p2s.enter_context(tc.tile_pool(name="pct", bufs=2, space="PSUM"))
                po_pool = p2s.enter_context(tc.tile_pool(name="po", bufs=2, space="PSUM"))

                ctx_q = {}   # st -> ctx tile
                ctxT_q = {}  # st -> ctxT tile

                def stage_num(st):
                    s0 = st * 128
                    pnums = [pnum_pool.tile([128, 2, 130], f32, tag=f"pnum{i}", name=f"pnum{i}") for i in range(2)]
                    for p in range(4):
                        nc.tensor.matmul(pnums[p // 2][:, p % 2, :], ET[:, p, s0:s0 + 128], kvbd[p], start=True, stop=True)
                    ctxs = p2.tile([128, OG], bf16, tag="ctx", name="ctxs")
                    for i in range(2):
                        pn4 = pnums[i].rearrange("p j (two c) -> p (j two) c", two=2)  # [128, 4, 65]
                        r4 = p2.tile([128, 4, 1], f32, tag="r", name="r4")
                        nc.vector.reciprocal(r4, pn4[:, :, 64:65])
                        ctx4 = ctxs[:, i * 256:(i + 1) * 256].rearrange("p (j c) -> p j c", c=64)
                        nc.vector.tensor_mul(ctx4, pn4[:, :, 0:64], r4.to_broadcast([128, 4, 64]))
                    ctx_q[st] = ctxs

                def stage_ctxT(st):
                    ctxs = ctx_q.pop(st)
                    pct = pct_pool.tile([128, 512], bf16, tag="pct", name="pct")
                    for eb in range(4):
                        nc.tensor.transpose(pct[:, eb * 128:(eb + 1) * 128], ctxs[:, eb * 128:(eb + 1) * 128], ident)
                    ctxT = p2.tile([128, 4, 128], bf16, tag="ctxT", name="ctxT")
                    nc.scalar.copy(out=ctxT, in_=pct.rearrange("p (j s) -> p j s", j=4))
                    ctxT_q[st] = ctxT

                def stage_oproj(st):
                    ctxT = ctxT_q.pop(st)
                    for half in range(2):
                        po = po_pool.tile([128, 512], f32, tag="po", name="po")
                        for eb in range(4):
                            nc.tensor.matmul(po, ctxT[:, eb, :],
                                             wo_sb[:, eb, half * 512:(half + 1) * 512],
                                             start=(eb == 0), stop=(eb == 3))
                        outsb = p2.tile([128, 512], bf16, tag=f"outsb{half}", name="outsb")
                        if half == 0:
                            nc.scalar.copy(out=outsb, in_=po)
                        else:
                            nc.vector.tensor_copy(outsb, po)
                        nc.sync.dma_start(out=out[st * 128:(st + 1) * 128, half * 512:(half + 1) * 512], in_=outsb)

                for st in range(NT):
                    stage_num(st)
                    if st >= 2:
                        stage_ctxT(st - 2)
                    if st >= 4:
                        stage_oproj(st - 4)
                for st in range(NT - 2, NT):
                    stage_ctxT(st)
                for st in range(NT - 4, NT):
                    stage_oproj(st)

    nc.compile()
    return nc


_LAST_RESULT = None


def _pack_st(x, dt_):
    # [S, D] f32 -> [NT*128, 1024] dt, row st*128+p col t*128+s_local = x[st*128+s, t*128+p]
    xr = x.reshape(NT, 128, 8, 128).transpose(0, 3, 2, 1)  # [st, p, t, s]
    return np.ascontiguousarray(xr.reshape(NT * 128, D)).astype(dt_)


def _pack_q(q, f8):
    # [S, D] f32 -> [NMAC*128, 4096] fp8, row a*128+p col t*512+s_local
    qr = q.reshape(NMAC, SM, 8, 128).transpose(0, 3, 2, 1)  # [a, p, t, s]
    return np.ascontiguousarray(qr.reshape(NMAC * 128, 8 * SM)).astype(f8)


def kernel(q, k, v, mask, Wq, bq, Wk, bk, Wv, bv, Wo, bo):
    global _LAST_RESULT
    import ml_dtypes
    from concourse.bass_utils import run_bass_kernel_spmd

    q = np.asarray(q, np.float32)
    k = np.asarray(k, np.float32)
    v = np.asarray(v, np.float32)
    mask = np.asarray(mask)
    Wq = np.asarray(Wq, np.float32)
    Wk = np.asarray(Wk, np.float32)
    Wv = np.asarray(Wv, np.float32)
    Wo = np.asarray(Wo, np.float32)
    bq = np.asarray(bq, np.float32)
    bk = np.asarray(bk, np.float32)
    bv = np.asarray(bv, np.float32)
    bo = np.asarray(bo, np.float32)

    nc = _build(bool(np.any(bk) or np.any(bv)))

    f8 = ml_dtypes.float8_e4m3
    bf = ml_dtypes.bfloat16
    xk_b = [_pack_st(k[b], f8) for b in range(B)]
    xv_b = [_pack_st(v[b], bf) for b in range(B)]
    xq_b = [_pack_q(q[b], f8) for b in range(B)]

    in_maps = []
    for core in range(NCORES):
        b, g = core // 2, core % 2
        sl = slice(g * OG, (g + 1) * OG)
        maskf = mask[b, 0, 0, :].astype(np.float32).reshape(NT, 128).T.copy()
        in_maps.append({
            "xkp": xk_b[b],
            "xvp": xv_b[b],
            "xqp": xq_b[b],
            "wqt": np.ascontiguousarray(Wq[sl, :].T).astype(f8),
            "wkt": np.ascontiguousarray(Wk[sl, :].T).astype(f8),
            "wvt": np.ascontiguousarray(Wv[sl, :].T).astype(bf),
            "wot": np.ascontiguousarray(Wo[:, sl].T).astype(bf),
            "bqs": np.ascontiguousarray((bq[sl] * SCALE).reshape(4, 128).T),
            "bk": bk[sl].reshape(1, OG).copy(),
            "bv": bv[sl].reshape(1, OG).copy(),
            "maskf": maskf,
        })

    res = run_bass_kernel_spmd(nc, in_maps, list(range(NCORES)))
    _LAST_RESULT = res

    outp = np.empty((B, S, D), np.float32)
    for b in range(B):
        o0 = res.results[2 * b]["out"].astype(np.float32).reshape(S, D)
        o1 = res.results[2 * b + 1]["out"].astype(np.float32).reshape(S, D)
        outp[b] = o0 + o1 + bo[None, :]
    return outp
